# revision 32
# baseline (speedup 1.0000x reference)
"""Trainium2 Bass kernel for nn_MemoryGraphBackprop (GNN message passing).

Strategy
--------
T=64 sequential steps over state [BS=2, N=1024, D=64].  The recurrence is
latency-bound and all operands (dense adjacency A = 2MB bf16, cc = 1MB,
state < 1MB) fit in SBUF, while an 8-core shard would need a per-step
all-gather of pm (>=5us collective floor x 64 sequential steps) that dwarfs
the compute.  So: ONE NeuronCore, fully SBUF-resident recurrence; the other
cores idle.

Math (per step t):
    r   = A @ pm  (+ cc_t into nodes < C)
    dt  = decay * (1 - eot[b,t])
    h'  = dt*h + (1-dt)*r
    pm' = tanh(prim * h')
With u := prim*h the update is
    u'  = dt*u + W2_t*r,   W2_t = (1-dt)*prim
In layout-2 ([(b,d) part, n free]) the eot factor is per-PARTITION, so when
decay is spatially uniform (decay_logit == const, the spec's fill) dt and
(1-dt) are per-partition scalars: s_b = dt*u and W2 are single tensor_scalar
ops off the critical path, and the cc inject is folded into s_b on DVE
(s_b[:, :C] += W2[:, :C]*cc_t) so the PE never touches it.  A general
fallback handles non-uniform decay.

Per step: PE runs 16 accumulating [K=128, M=128, FD=512] bf16 matmuls
(half-0 of the n-range fully before half-1, so half-0's elementwise chain
overlaps half-1's matmuls) then 8 transpose-mode matmuls of u' back to
layout-1.  The psum chain u'_q = W2_q*r_q + s_b_q runs on DVE in FD=256
quarters; ACT fuses tanh into the PSUM->SBUF copy of each transposed
quarter (producing pm = the next step's stationary operand) and emits the
[C, BS*D] output slice in fp32.  A block of dummy identity matmuls warms
the PE HAM clock-gate while the input DMAs land.

Measured on trn2 (axon): ~320-325us HW exec, rel_l2 vs the fp32 jax
reference ~3.9e-3 (bf16 operands; fp32 psum accumulation throughout).

Layouts:
  l2 (state u, psum r):  [128 part = b*64+d, 1024 free = n]
  l1 (pm, matmul lhsT):  [128 part = n%128, free = (n//128)*128 + b*64 + d]
"""

import sys

if "/opt/trn_rl_repo" not in sys.path:
    sys.path.insert(0, "/opt/trn_rl_repo")

import numpy as np

import concourse.bass as bass
import concourse.mybir as mybir
import concourse.tile as tile
from concourse import bass_utils

BS, T, C, D = 2, 64, 64, 64
N = 1024
NT = N // 128  # 8 node chunks
P = 128        # BS*D partitions in layout-2
NQ = 4         # chain quarters
QW = N // NQ   # 256

F32 = mybir.dt.float32
BF16 = mybir.dt.bfloat16

# ---------------------------------------------------------------------------
# Workaround: this container's walrus accepts only ONE sync-wait per
# instruction.  (1) Tile's tail drain attaches one wait per live semaphore —
# split across multiple drains.  (2) Any multi-wait instruction gets its
# extra waits hoisted onto InstEventSemaphore carriers just before it.
# ---------------------------------------------------------------------------
from concourse.vector_clock import ScopedClock  # noqa: E402


def _patched_drain_and_barrier(self, tick_clock, wait_clock):
    drain_inst = self.nc.sync.drain()
    wait_clock.add_sem_waits(
        drain_inst.ins, ScopedClock({None: tick_clock.global_clock})
    )
    si = drain_inst.ins.sync_info
    if si is not None and si.on_wait is not None and len(si.on_wait) > 1:
        waits = list(si.on_wait)
        drain_inst.ins.sync_info = mybir.SyncInfo(
            on_wait=[waits[0]], on_update=si.on_update
        )
        for w in waits[1:]:
            d2 = self.nc.sync.drain()
            d2.ins.sync_info = mybir.SyncInfo(on_wait=[w], on_update=[])

    self.nc.all_engine_barrier()
    assert self.sems is not None
    popped = self.nc._tile_sem_poison_stack.pop()
    assert popped is self._sem_poison
    self.nc.clear_and_free_semaphores(list(self.sems.allocated().values()))
    self.nc.all_engine_barrier()


tile.TileContext._drain_and_barrier = _patched_drain_and_barrier


def _split_multi_waits(nc):
    n_carriers = 0
    for bb in nc.m.functions[0].blocks:
        insts = list(bb.instructions)
        out = []
        changed = False
        for inst in insts:
            si = inst.sync_info
            if si is not None and si.on_wait is not None and len(si.on_wait) > 1:
                waits = list(si.on_wait)
                for w in waits[:-1]:
                    n_carriers += 1
                    carrier = mybir.InstEventSemaphore(
                        name=f"waitsplit-{n_carriers}", ins=[], outs=[]
                    )
                    carrier.engine = inst.engine
                    carrier.sync_info = mybir.SyncInfo(on_wait=[w], on_update=[])
                    out.append(carrier)
                inst.sync_info = mybir.SyncInfo(
                    on_wait=[waits[-1]], on_update=si.on_update
                )
                changed = True
            out.append(inst)
        if changed:
            bb.instructions = out
    return n_carriers


# ---------------------------------------------------------------------------
# Host-side input massaging (layouts, scatter into dense A, norms, sigmoid).
# ---------------------------------------------------------------------------
def _prep_host(inputs):
    import ml_dtypes

    bf16 = ml_dtypes.bfloat16

    cc = np.asarray(inputs["cc_signals"], dtype=np.float32)       # [B,T,C,D]
    eot = np.asarray(inputs["eot_mask"]).astype(bool)             # [B,T]
    idx = np.asarray(inputs["conn_indices"]).astype(np.int64)     # [N,K]
    cmask = np.asarray(inputs["conn_mask"]).astype(np.float32)    # [N,K]
    prim = np.asarray(inputs["primitives"], dtype=np.float32)     # [N,D]
    w = np.asarray(inputs["conn_weights"], dtype=np.float32)      # [N,K]
    dlog = np.asarray(inputs["decay_logit"], dtype=np.float32)    # [N]
    h0 = np.asarray(inputs["h0"], dtype=np.float32)               # [B,N,D]
    pm0 = np.asarray(inputs["prev_msg0"], dtype=np.float32)       # [B,N,D]

    # dense adjacency, transposed for the layout-2 matmul (rhs[m, n] = A[n, m])
    A = np.zeros((N, N), dtype=np.float32)
    np.add.at(A, (np.arange(N)[:, None], idx), w * cmask)
    At = np.ascontiguousarray(A.T)                                # [m, n]
    at_host = At.reshape(NT, 128, N).transpose(1, 0, 2).reshape(128, NT * N)

    # L2-normalized cc in layout-1 slabs: [c, t*128 + b*64 + d]
    nrm = np.maximum(np.linalg.norm(cc, axis=-1, keepdims=True), 1e-8)
    ccn = (cc / nrm).astype(np.float32)
    cc_host = np.ascontiguousarray(ccn.transpose(2, 1, 0, 3).reshape(C, T * P))
    # layout-2 cc: [b*64+d partitions, t*64 + n(<C) free]
    cc2_host = np.ascontiguousarray(
        ccn.transpose(0, 3, 1, 2).reshape(P, T * C)
    )

    decay = (1.0 / (1.0 + np.exp(-dlog.astype(np.float64)))).astype(np.float32)
    uniform = bool(np.all(decay == decay[0]))

    prim_l2 = np.ascontiguousarray(np.tile(prim.T, (BS, 1)))      # [128, N]
    fmat = np.repeat((~eot).astype(np.float32), D, axis=0)        # [128, T]

    h0_l2 = h0.transpose(0, 2, 1).reshape(P, N)                   # [b*64+d, n]
    u0 = np.ascontiguousarray(prim_l2 * h0_l2)

    pm0_l1 = np.ascontiguousarray(
        pm0.reshape(BS, NT, 128, D).transpose(2, 1, 0, 3).reshape(128, NT * P)
    )

    host = {
        "at": at_host.astype(bf16),
        "cc2": cc2_host.astype(bf16),
        "prim": prim_l2.astype(bf16),
        "u0": u0.astype(bf16),
        "pm0": pm0_l1.astype(bf16),
    }
    if uniform:
        g = decay[0] * fmat                                       # [128, T]
        host["gmat"] = np.ascontiguousarray(g.astype(np.float32))
        host["h1g"] = np.ascontiguousarray((1.0 - g).astype(np.float32))
    else:
        dec_l2 = np.ascontiguousarray(np.broadcast_to(decay[None, :], (P, N)))
        host["dec"] = dec_l2.astype(bf16)
        host["dp"] = (prim_l2 * decay[None, :]).astype(bf16)
        host["fmat"] = np.ascontiguousarray(fmat.astype(np.float32))
    return host, uniform


# ---------------------------------------------------------------------------
# Device kernel
# ---------------------------------------------------------------------------
def _build_bass(uniform):
    nc = bass.Bass("TRN2", target_bir_lowering=False, debug=False)

    at_d = nc.dram_tensor("at", [128, NT * N], BF16, kind="ExternalInput")
    cc2_d = nc.dram_tensor("cc2", [P, T * C], BF16, kind="ExternalInput")
    prim_d = nc.dram_tensor("prim", [P, N], BF16, kind="ExternalInput")
    u0_d = nc.dram_tensor("u0", [P, N], BF16, kind="ExternalInput")
    pm0_d = nc.dram_tensor("pm0", [128, NT * P], BF16, kind="ExternalInput")
    out_d = nc.dram_tensor("out", [T, C, P], F32, kind="ExternalOutput")
    if uniform:
        g_d = nc.dram_tensor("gmat", [P, T], F32, kind="ExternalInput")
        h1g_d = nc.dram_tensor("h1g", [P, T], F32, kind="ExternalInput")
    else:
        dec_d = nc.dram_tensor("dec", [P, N], BF16, kind="ExternalInput")
        dp_d = nc.dram_tensor("dp", [P, N], BF16, kind="ExternalInput")
        f_d = nc.dram_tensor("fmat", [P, T], F32, kind="ExternalInput")

    Tanh = mybir.ActivationFunctionType.Tanh

    with tile.TileContext(nc) as tc:
        with (
            tc.tile_pool(name="consts", bufs=1) as consts,
            tc.tile_pool(name="state", bufs=3) as state,
            tc.tile_pool(name="tmp", bufs=3) as tmp,
            tc.tile_pool(name="psr", bufs=2, space="PSUM") as psr,
            tc.tile_pool(name="ptp", bufs=4, space="PSUM") as ptp,
        ):
            # --- load state + small constants first so step-0 deps clear
            # --- early, then the big A / cc slabs ---
            id128_sb = consts.tile([128, 128], BF16)
            from concourse.masks import make_identity
            make_identity(nc, id128_sb[:])

            # HAM warm-up: ~64 dummy matmuls on the identity keep the PE
            # activity monitor at full clock while the input DMAs land.
            warm_ps = psr.tile([128, 128], F32, tag="ps0", name="warm_ps")
            for i in range(64):
                nc.tensor.matmul(
                    warm_ps[:], id128_sb[:], id128_sb[:],
                    start=(i == 0), stop=(i == 63), skip_group_check=True,
                )

            u = [
                state.tile([P, 512], BF16, tag="u0h", name="u_lo"),
                state.tile([P, 512], BF16, tag="u1h", name="u_hi"),
            ]
            pm = [
                state.tile([128, 512], BF16, tag="pm0h", name="pm_lo"),
                state.tile([128, 512], BF16, tag="pm1h", name="pm_hi"),
            ]
            for h in range(2):
                nc.sync.dma_start(out=u[h][:], in_=u0_d.ap()[:, h * 512:(h + 1) * 512])
                nc.sync.dma_start(out=pm[h][:], in_=pm0_d.ap()[:, h * 512:(h + 1) * 512])
            at_sb = consts.tile([128, NT * N], BF16)
            for h in range(2):
                for m in range(NT):
                    sl = slice(m * N + h * 512, m * N + (h + 1) * 512)
                    nc.sync.dma_start(out=at_sb[:, sl], in_=at_d.ap()[:, sl])
            prim_sb = consts.tile([P, N], BF16)
            nc.sync.dma_start(out=prim_sb[:], in_=prim_d.ap()[:])
            if uniform:
                g_sb = consts.tile([P, T], F32)
                nc.sync.dma_start(out=g_sb[:], in_=g_d.ap()[:])
                h1g_sb = consts.tile([P, T], F32)
                nc.sync.dma_start(out=h1g_sb[:], in_=h1g_d.ap()[:])
            else:
                dec_sb = consts.tile([P, N], BF16)
                nc.sync.dma_start(out=dec_sb[:], in_=dec_d.ap()[:])
                dp_sb = consts.tile([P, N], BF16)
                nc.sync.dma_start(out=dp_sb[:], in_=dp_d.ap()[:])
                f_sb = consts.tile([P, T], F32)
                nc.sync.dma_start(out=f_sb[:], in_=f_d.ap()[:])
            cc2_sb = consts.tile([P, T * C], BF16)
            for q in range(4):
                s = slice(q * (T * C) // 4, (q + 1) * (T * C) // 4)
                nc.sync.dma_start(out=cc2_sb[:, s], in_=cc2_d.ap()[:, s])

            for t in range(T):
                # ---- off-critical-path per-step tensors: s_b = dt*u,
                # ---- w2 = (1-dt)*prim  (per half to keep deps narrow)
                sb_t = [
                    tmp.tile([P, 512], BF16, tag="sb0", name="sb_lo"),
                    tmp.tile([P, 512], BF16, tag="sb1", name="sb_hi"),
                ]
                w2 = tmp.tile([P, N], BF16, tag="w2")
                if uniform:
                    gt = g_sb[:, t:t + 1]
                    nc.vector.tensor_scalar_mul(sb_t[0][:], u[0][:], gt)
                    nc.vector.tensor_scalar_mul(sb_t[1][:], u[1][:], gt)
                    nc.vector.tensor_scalar_mul(
                        w2[:], prim_sb[:], h1g_sb[:, t:t + 1]
                    )
                else:
                    ft = f_sb[:, t:t + 1]
                    w0 = tmp.tile([P, N], BF16, tag="w0")
                    nc.vector.tensor_scalar_mul(w0[:], dec_sb[:], ft)
                    nc.vector.tensor_mul(sb_t[0][:], u[0][:], w0[:, 0:512])
                    nc.vector.tensor_mul(sb_t[1][:], u[1][:], w0[:, 512:1024])
                    w1 = tmp.tile([P, N], BF16, tag="w1")
                    nc.vector.tensor_scalar_mul(w1[:], dp_sb[:], ft)
                    nc.vector.tensor_sub(w2[:], prim_sb[:], w1[:])
                # cc inject: sb[:, :C] += w2[:, :C] * cc_t  (adds (1-dt)*cc,
                # folded under W2 so the PE matmul group skips it)
                cw = tmp.tile([P, C], BF16, tag="cw")
                nc.vector.tensor_mul(
                    cw[:], w2[:, 0:C], cc2_sb[:, t * C:(t + 1) * C]
                )
                nc.vector.tensor_add(sb_t[0][:, 0:C], sb_t[0][:, 0:C], cw[:])

                # ---- matmuls: r = A @ pm, layout-2 psum ----
                # Half-0 fully before half-1 so half-0's chain overlaps
                # half-1's matmuls on PE.
                ps = [
                    psr.tile([P, 512], F32, tag="ps0", name="ps0"),
                    psr.tile([P, 512], F32, tag="ps1", name="ps1"),
                ]
                un = [
                    state.tile([P, 512], BF16, tag="u0h", name="un_lo"),
                    state.tile([P, 512], BF16, tag="u1h", name="un_hi"),
                ]
                pmn = [
                    state.tile([128, 512], BF16, tag="pm0h", name="pmn_lo"),
                    state.tile([128, 512], BF16, tag="pm1h", name="pmn_hi"),
                ]
                pts = [
                    ptp.tile([128, QW], BF16, tag="pt", name="pt")
                    for _ in range(NQ)
                ]
                for h in range(2):
                    for m in range(NT):
                        nc.tensor.matmul(
                            ps[h][:],
                            pm[m // 4][:, (m % 4) * P:(m % 4 + 1) * P],
                            at_sb[:, m * N + h * 512: m * N + (h + 1) * 512],
                            start=(m == 0),
                            stop=(m == NT - 1),
                            skip_group_check=True,
                        )
                    # chain (DVE) per quarter; emitted right after the half's
                    # matmuls so DVE starts while the other half's MMs run.
                    for hq in range(2):
                        q = h * 2 + hq
                        if t == T - 1 and q > 0:
                            continue  # last step: only chunk 0 reaches out
                        psl = slice(hq * QW, (hq + 1) * QW)
                        x = tmp.tile([P, QW], BF16, tag=f"x{q}", name="x")
                        nc.vector.tensor_mul(x[:], ps[h][:, psl], w2[:, q * QW:(q + 1) * QW])
                        nc.vector.tensor_add(
                            un[h][:, psl], x[:], sb_t[h][:, psl]
                        )

                # transposes + fused tanh per quarter, inline: the
                # scheduler interleaves them into MM-stream slack (measured
                # faster than one contiguous transpose cluster).
                out_sb = tmp.tile([C, P], F32, tag="out_sb")
                for q in range(4):
                    if t == T - 1 and q > 0:
                        continue  # last step: only chunk 0 reaches out
                    h, hq = divmod(q, 2)
                    for j in range(2):
                        if t == T - 1 and (hq * 2 + j) > 0:
                            continue
                        mloc = hq * 2 + j
                        nc.tensor.transpose(
                            pts[q][:, j * 128:(j + 1) * 128],
                            un[h][:, mloc * 128:(mloc + 1) * 128],
                            id128_sb[:],
                        )
                    if t < T - 1:
                        nc.scalar.activation(
                            pmn[h][:, hq * QW:(hq + 1) * QW], pts[q][:], Tanh
                        )
                    if q == 0:
                        # fp32 output slice right after tanh_q0 so pts[0]'s
                        # psum slot frees early for the next step.
                        nc.scalar.activation(out_sb[:], pts[0][0:C, 0:P], Tanh)
                nc.sync.dma_start(out=out_d.ap()[t], in_=out_sb[:])

                u, pm = un, pmn

    _split_multi_waits(nc)
    return nc


RUN_KWARGS: dict = {}
_BUILT: dict = {}


def _get_built(uniform):
    if uniform not in _BUILT:
        _BUILT[uniform] = _build_bass(uniform)
    return _BUILT[uniform]


def kernel(**inputs) -> np.ndarray:
    host, uniform = _prep_host(inputs)
    nc = _get_built(uniform)
    res = bass_utils.run_bass_kernel_spmd(nc, [host], core_ids=[0], **RUN_KWARGS)
    kernel.last_result = res
    out_dev = res.results[0]["out"]                               # [T, C, 128]
    out = out_dev.reshape(T, C, BS, D).transpose(2, 0, 1, 3)      # [B,T,C,D]
    return np.ascontiguousarray(out)


if __name__ == "__main__":
    print("standalone smoke: building bass module (uniform decay path)...")
    _get_built(True)
    print("built ok")


# revision 33
# speedup vs baseline: 1.0017x; 1.0017x over previous
"""Trainium2 Bass kernel for nn_MemoryGraphBackprop (GNN message passing).

Strategy
--------
T=64 sequential steps over state [BS=2, N=1024, D=64].  The recurrence is
latency-bound and all operands (dense adjacency A = 2MB bf16, cc = 1MB,
state < 1MB) fit in SBUF, while an 8-core shard would need a per-step
all-gather of pm (>=5us collective floor x 64 sequential steps) that dwarfs
the compute.  So: ONE NeuronCore, fully SBUF-resident recurrence; the other
cores idle.

Math (per step t):
    r   = A @ pm  (+ cc_t into nodes < C)
    dt  = decay * (1 - eot[b,t])
    h'  = dt*h + (1-dt)*r
    pm' = tanh(prim * h')
With u := prim*h the update is
    u'  = dt*u + W2_t*r,   W2_t = (1-dt)*prim
In layout-2 ([(b,d) part, n free]) the eot factor is per-PARTITION, so when
decay is spatially uniform (decay_logit == const, the spec's fill) dt and
(1-dt) are per-partition scalars: s_b = dt*u and W2 are single tensor_scalar
ops off the critical path, and the cc inject is folded into s_b on DVE
(s_b[:, :C] += W2[:, :C]*cc_t) so the PE never touches it.  A general
fallback handles non-uniform decay.

Per step: PE runs 16 accumulating [K=128, M=128, FD=512] bf16 matmuls
(half-0 of the n-range fully before half-1, so half-0's elementwise chain
overlaps half-1's matmuls) then 8 transpose-mode matmuls of u' back to
layout-1.  The psum chain u'_q = W2_q*r_q + s_b_q runs on DVE in FD=256
quarters; ACT fuses tanh into the PSUM->SBUF copy of each transposed
quarter (producing pm = the next step's stationary operand) and emits the
[C, BS*D] output slice in fp32.  A block of dummy identity matmuls warms
the PE HAM clock-gate while the input DMAs land.

Measured on trn2 (axon): ~320-325us HW exec, rel_l2 vs the fp32 jax
reference ~3.9e-3 (bf16 operands; fp32 psum accumulation throughout).

Layouts:
  l2 (state u, psum r):  [128 part = b*64+d, 1024 free = n]
  l1 (pm, matmul lhsT):  [128 part = n%128, free = (n//128)*128 + b*64 + d]
"""

import sys

if "/opt/trn_rl_repo" not in sys.path:
    sys.path.insert(0, "/opt/trn_rl_repo")

import numpy as np

import concourse.bass as bass
import concourse.mybir as mybir
import concourse.tile as tile
from concourse import bass_utils

BS, T, C, D = 2, 64, 64, 64
N = 1024
NT = N // 128  # 8 node chunks
P = 128        # BS*D partitions in layout-2
NQ = 4         # chain quarters
QW = N // NQ   # 256

F32 = mybir.dt.float32
BF16 = mybir.dt.bfloat16

# ---------------------------------------------------------------------------
# Workaround: this container's walrus accepts only ONE sync-wait per
# instruction.  (1) Tile's tail drain attaches one wait per live semaphore —
# split across multiple drains.  (2) Any multi-wait instruction gets its
# extra waits hoisted onto InstEventSemaphore carriers just before it.
# ---------------------------------------------------------------------------
from concourse.vector_clock import ScopedClock  # noqa: E402


def _patched_drain_and_barrier(self, tick_clock, wait_clock):
    drain_inst = self.nc.sync.drain()
    wait_clock.add_sem_waits(
        drain_inst.ins, ScopedClock({None: tick_clock.global_clock})
    )
    si = drain_inst.ins.sync_info
    if si is not None and si.on_wait is not None and len(si.on_wait) > 1:
        waits = list(si.on_wait)
        drain_inst.ins.sync_info = mybir.SyncInfo(
            on_wait=[waits[0]], on_update=si.on_update
        )
        for w in waits[1:]:
            d2 = self.nc.sync.drain()
            d2.ins.sync_info = mybir.SyncInfo(on_wait=[w], on_update=[])

    self.nc.all_engine_barrier()
    assert self.sems is not None
    popped = self.nc._tile_sem_poison_stack.pop()
    assert popped is self._sem_poison
    self.nc.clear_and_free_semaphores(list(self.sems.allocated().values()))
    self.nc.all_engine_barrier()


tile.TileContext._drain_and_barrier = _patched_drain_and_barrier


def _split_multi_waits(nc):
    n_carriers = 0
    for bb in nc.m.functions[0].blocks:
        insts = list(bb.instructions)
        out = []
        changed = False
        for inst in insts:
            si = inst.sync_info
            if si is not None and si.on_wait is not None and len(si.on_wait) > 1:
                waits = list(si.on_wait)
                for w in waits[:-1]:
                    n_carriers += 1
                    carrier = mybir.InstEventSemaphore(
                        name=f"waitsplit-{n_carriers}", ins=[], outs=[]
                    )
                    carrier.engine = inst.engine
                    carrier.sync_info = mybir.SyncInfo(on_wait=[w], on_update=[])
                    out.append(carrier)
                inst.sync_info = mybir.SyncInfo(
                    on_wait=[waits[-1]], on_update=si.on_update
                )
                changed = True
            out.append(inst)
        if changed:
            bb.instructions = out
    return n_carriers


# ---------------------------------------------------------------------------
# Host-side input massaging (layouts, scatter into dense A, norms, sigmoid).
# ---------------------------------------------------------------------------
def _prep_host(inputs):
    import ml_dtypes

    bf16 = ml_dtypes.bfloat16

    cc = np.asarray(inputs["cc_signals"], dtype=np.float32)       # [B,T,C,D]
    eot = np.asarray(inputs["eot_mask"]).astype(bool)             # [B,T]
    idx = np.asarray(inputs["conn_indices"]).astype(np.int64)     # [N,K]
    cmask = np.asarray(inputs["conn_mask"]).astype(np.float32)    # [N,K]
    prim = np.asarray(inputs["primitives"], dtype=np.float32)     # [N,D]
    w = np.asarray(inputs["conn_weights"], dtype=np.float32)      # [N,K]
    dlog = np.asarray(inputs["decay_logit"], dtype=np.float32)    # [N]
    h0 = np.asarray(inputs["h0"], dtype=np.float32)               # [B,N,D]
    pm0 = np.asarray(inputs["prev_msg0"], dtype=np.float32)       # [B,N,D]

    # dense adjacency, transposed for the layout-2 matmul (rhs[m, n] = A[n, m])
    A = np.zeros((N, N), dtype=np.float32)
    np.add.at(A, (np.arange(N)[:, None], idx), w * cmask)
    At = np.ascontiguousarray(A.T)                                # [m, n]
    at_host = At.reshape(NT, 128, N).transpose(1, 0, 2).reshape(128, NT * N)

    # L2-normalized cc in layout-1 slabs: [c, t*128 + b*64 + d]
    nrm = np.maximum(np.linalg.norm(cc, axis=-1, keepdims=True), 1e-8)
    ccn = (cc / nrm).astype(np.float32)
    cc_host = np.ascontiguousarray(ccn.transpose(2, 1, 0, 3).reshape(C, T * P))
    # layout-2 cc: [b*64+d partitions, t*64 + n(<C) free]
    cc2_host = np.ascontiguousarray(
        ccn.transpose(0, 3, 1, 2).reshape(P, T * C)
    )

    decay = (1.0 / (1.0 + np.exp(-dlog.astype(np.float64)))).astype(np.float32)
    uniform = bool(np.all(decay == decay[0]))

    prim_l2 = np.ascontiguousarray(np.tile(prim.T, (BS, 1)))      # [128, N]
    fmat = np.repeat((~eot).astype(np.float32), D, axis=0)        # [128, T]

    h0_l2 = h0.transpose(0, 2, 1).reshape(P, N)                   # [b*64+d, n]
    u0 = np.ascontiguousarray(prim_l2 * h0_l2)

    pm0_l1 = np.ascontiguousarray(
        pm0.reshape(BS, NT, 128, D).transpose(2, 1, 0, 3).reshape(128, NT * P)
    )

    host = {
        "at": at_host.astype(bf16),
        "cc2": cc2_host.astype(bf16),
        "prim": prim_l2.astype(bf16),
        "u0": u0.astype(bf16),
        "pm0": pm0_l1.astype(bf16),
    }
    if uniform:
        g = decay[0] * fmat                                       # [128, T]
        host["gmat"] = np.ascontiguousarray(g.astype(np.float32))
        host["h1g"] = np.ascontiguousarray((1.0 - g).astype(np.float32))
    else:
        dec_l2 = np.ascontiguousarray(np.broadcast_to(decay[None, :], (P, N)))
        host["dec"] = dec_l2.astype(bf16)
        host["dp"] = (prim_l2 * decay[None, :]).astype(bf16)
        host["fmat"] = np.ascontiguousarray(fmat.astype(np.float32))
    return host, uniform


# ---------------------------------------------------------------------------
# Device kernel
# ---------------------------------------------------------------------------
def _build_bass(uniform):
    nc = bass.Bass("TRN2", target_bir_lowering=False, debug=False)

    at_d = nc.dram_tensor("at", [128, NT * N], BF16, kind="ExternalInput")
    cc2_d = nc.dram_tensor("cc2", [P, T * C], BF16, kind="ExternalInput")
    prim_d = nc.dram_tensor("prim", [P, N], BF16, kind="ExternalInput")
    u0_d = nc.dram_tensor("u0", [P, N], BF16, kind="ExternalInput")
    pm0_d = nc.dram_tensor("pm0", [128, NT * P], BF16, kind="ExternalInput")
    out_d = nc.dram_tensor("out", [T, C, P], F32, kind="ExternalOutput")
    if uniform:
        g_d = nc.dram_tensor("gmat", [P, T], F32, kind="ExternalInput")
        h1g_d = nc.dram_tensor("h1g", [P, T], F32, kind="ExternalInput")
    else:
        dec_d = nc.dram_tensor("dec", [P, N], BF16, kind="ExternalInput")
        dp_d = nc.dram_tensor("dp", [P, N], BF16, kind="ExternalInput")
        f_d = nc.dram_tensor("fmat", [P, T], F32, kind="ExternalInput")

    Tanh = mybir.ActivationFunctionType.Tanh

    with tile.TileContext(nc) as tc:
        with (
            tc.tile_pool(name="consts", bufs=1) as consts,
            tc.tile_pool(name="state", bufs=3) as state,
            tc.tile_pool(name="tmp", bufs=3) as tmp,
            tc.tile_pool(name="psr", bufs=3, space="PSUM") as psr,
            tc.tile_pool(name="ptp", bufs=2, space="PSUM") as ptp,
        ):
            # --- load state + small constants first so step-0 deps clear
            # --- early, then the big A / cc slabs ---
            id128_sb = consts.tile([128, 128], BF16)
            from concourse.masks import make_identity
            make_identity(nc, id128_sb[:])

            # HAM warm-up: ~64 dummy matmuls on the identity keep the PE
            # activity monitor at full clock while the input DMAs land.
            warm_ps = psr.tile([128, 128], F32, tag="ps0", name="warm_ps")
            for i in range(64):
                nc.tensor.matmul(
                    warm_ps[:], id128_sb[:], id128_sb[:],
                    start=(i == 0), stop=(i == 63), skip_group_check=True,
                )

            u = [
                state.tile([P, 512], BF16, tag="u0h", name="u_lo"),
                state.tile([P, 512], BF16, tag="u1h", name="u_hi"),
            ]
            pm = [
                state.tile([128, 512], BF16, tag="pm0h", name="pm_lo"),
                state.tile([128, 512], BF16, tag="pm1h", name="pm_hi"),
            ]
            for h in range(2):
                nc.sync.dma_start(out=u[h][:], in_=u0_d.ap()[:, h * 512:(h + 1) * 512])
                nc.sync.dma_start(out=pm[h][:], in_=pm0_d.ap()[:, h * 512:(h + 1) * 512])
            at_sb = consts.tile([128, NT * N], BF16)
            for h in range(2):
                for m in range(NT):
                    sl = slice(m * N + h * 512, m * N + (h + 1) * 512)
                    nc.sync.dma_start(out=at_sb[:, sl], in_=at_d.ap()[:, sl])
            prim_sb = consts.tile([P, N], BF16)
            nc.sync.dma_start(out=prim_sb[:], in_=prim_d.ap()[:])
            if uniform:
                g_sb = consts.tile([P, T], F32)
                nc.sync.dma_start(out=g_sb[:], in_=g_d.ap()[:])
                h1g_sb = consts.tile([P, T], F32)
                nc.sync.dma_start(out=h1g_sb[:], in_=h1g_d.ap()[:])
            else:
                dec_sb = consts.tile([P, N], BF16)
                nc.sync.dma_start(out=dec_sb[:], in_=dec_d.ap()[:])
                dp_sb = consts.tile([P, N], BF16)
                nc.sync.dma_start(out=dp_sb[:], in_=dp_d.ap()[:])
                f_sb = consts.tile([P, T], F32)
                nc.sync.dma_start(out=f_sb[:], in_=f_d.ap()[:])
            cc2_sb = consts.tile([P, T * C], BF16)
            for q in range(4):
                s = slice(q * (T * C) // 4, (q + 1) * (T * C) // 4)
                nc.sync.dma_start(out=cc2_sb[:, s], in_=cc2_d.ap()[:, s])

            for t in range(T):
                # ---- off-critical-path per-step tensors: s_b = dt*u,
                # ---- w2 = (1-dt)*prim  (per half to keep deps narrow)
                sb_t = [
                    tmp.tile([P, 512], BF16, tag="sb0", name="sb_lo"),
                    tmp.tile([P, 512], BF16, tag="sb1", name="sb_hi"),
                ]
                w2 = tmp.tile([P, N], BF16, tag="w2")
                if uniform:
                    gt = g_sb[:, t:t + 1]
                    nc.vector.tensor_scalar_mul(sb_t[0][:], u[0][:], gt)
                    nc.vector.tensor_scalar_mul(sb_t[1][:], u[1][:], gt)
                    nc.vector.tensor_scalar_mul(
                        w2[:], prim_sb[:], h1g_sb[:, t:t + 1]
                    )
                else:
                    ft = f_sb[:, t:t + 1]
                    w0 = tmp.tile([P, N], BF16, tag="w0")
                    nc.vector.tensor_scalar_mul(w0[:], dec_sb[:], ft)
                    nc.vector.tensor_mul(sb_t[0][:], u[0][:], w0[:, 0:512])
                    nc.vector.tensor_mul(sb_t[1][:], u[1][:], w0[:, 512:1024])
                    w1 = tmp.tile([P, N], BF16, tag="w1")
                    nc.vector.tensor_scalar_mul(w1[:], dp_sb[:], ft)
                    nc.vector.tensor_sub(w2[:], prim_sb[:], w1[:])
                # cc inject: sb[:, :C] += w2[:, :C] * cc_t  (adds (1-dt)*cc,
                # folded under W2 so the PE matmul group skips it)
                cw = tmp.tile([P, C], BF16, tag="cw")
                nc.vector.tensor_mul(
                    cw[:], w2[:, 0:C], cc2_sb[:, t * C:(t + 1) * C]
                )
                nc.vector.tensor_add(sb_t[0][:, 0:C], sb_t[0][:, 0:C], cw[:])

                # ---- matmuls: r = A @ pm, layout-2 psum ----
                # Half-0 fully before half-1 so half-0's chain overlaps
                # half-1's matmuls on PE.
                ps = [
                    psr.tile([P, 512], F32, tag="ps0", name="ps0"),
                    psr.tile([P, 512], F32, tag="ps1", name="ps1"),
                ]
                un = [
                    state.tile([P, 512], BF16, tag="u0h", name="un_lo"),
                    state.tile([P, 512], BF16, tag="u1h", name="un_hi"),
                ]
                pmn = [
                    state.tile([128, 512], BF16, tag="pm0h", name="pmn_lo"),
                    state.tile([128, 512], BF16, tag="pm1h", name="pmn_hi"),
                ]
                pts = [
                    ptp.tile([128, QW], BF16, tag="pt", name="pt")
                    for _ in range(NQ)
                ]
                for h in range(2):
                    for m in range(NT):
                        nc.tensor.matmul(
                            ps[h][:],
                            pm[m // 4][:, (m % 4) * P:(m % 4 + 1) * P],
                            at_sb[:, m * N + h * 512: m * N + (h + 1) * 512],
                            start=(m == 0),
                            stop=(m == NT - 1),
                            skip_group_check=True,
                        )
                    # chain (DVE) per quarter; emitted right after the half's
                    # matmuls so DVE starts while the other half's MMs run.
                    for hq in range(2):
                        q = h * 2 + hq
                        if t == T - 1 and q > 0:
                            continue  # last step: only chunk 0 reaches out
                        psl = slice(hq * QW, (hq + 1) * QW)
                        x = tmp.tile([P, QW], BF16, tag=f"x{q}", name="x")
                        nc.vector.tensor_mul(x[:], ps[h][:, psl], w2[:, q * QW:(q + 1) * QW])
                        nc.vector.tensor_add(
                            un[h][:, psl], x[:], sb_t[h][:, psl]
                        )

                # transposes + fused tanh per quarter, inline: the
                # scheduler interleaves them into MM-stream slack (measured
                # faster than one contiguous transpose cluster).
                out_sb = tmp.tile([C, P], F32, tag="out_sb")
                for q in range(4):
                    if t == T - 1 and q > 0:
                        continue  # last step: only chunk 0 reaches out
                    h, hq = divmod(q, 2)
                    for j in range(2):
                        if t == T - 1 and (hq * 2 + j) > 0:
                            continue
                        mloc = hq * 2 + j
                        nc.tensor.transpose(
                            pts[q][:, j * 128:(j + 1) * 128],
                            un[h][:, mloc * 128:(mloc + 1) * 128],
                            id128_sb[:],
                        )
                    if t < T - 1:
                        nc.scalar.activation(
                            pmn[h][:, hq * QW:(hq + 1) * QW], pts[q][:], Tanh
                        )
                    if q == 0:
                        # fp32 output slice right after tanh_q0 so pts[0]'s
                        # psum slot frees early for the next step.
                        nc.scalar.activation(out_sb[:], pts[0][0:C, 0:P], Tanh)
                nc.sync.dma_start(out=out_d.ap()[t], in_=out_sb[:])

                u, pm = un, pmn

    _split_multi_waits(nc)
    return nc


RUN_KWARGS: dict = {}
_BUILT: dict = {}


def _get_built(uniform):
    if uniform not in _BUILT:
        _BUILT[uniform] = _build_bass(uniform)
    return _BUILT[uniform]


def kernel(**inputs) -> np.ndarray:
    host, uniform = _prep_host(inputs)
    nc = _get_built(uniform)
    res = bass_utils.run_bass_kernel_spmd(nc, [host], core_ids=[0], **RUN_KWARGS)
    kernel.last_result = res
    out_dev = res.results[0]["out"]                               # [T, C, 128]
    out = out_dev.reshape(T, C, BS, D).transpose(2, 0, 1, 3)      # [B,T,C,D]
    return np.ascontiguousarray(out)


if __name__ == "__main__":
    print("standalone smoke: building bass module (uniform decay path)...")
    _get_built(True)
    print("built ok")
